# revision 1
# baseline (speedup 1.0000x reference)
"""Single-head attention (b=4, s=4096, d=1024, h=128) on 8 Trainium2 NeuronCores.

Sharding: data-parallel over batch x query-halves -> 8 independent cores
(core c handles batch c//2, query rows [hq*2048, (hq+1)*2048) with hq = c%2).
K/V work is replicated per batch pair; no collectives.

Host prep per core: x[b].T (d-major), with the sequence columns rotated so the
core's 2048 query rows come first (softmax over keys is permutation-invariant,
so K/V key order doesn't matter). The softmax scale 1/sqrt(h) is folded into
Wq. The kernel returns outT [h, 2048] per core; the host transposes back.

Device kernel (per core, all layouts feature-on-partitions, float32r matmuls):
  qT = wq.T @ xT          [128h, 2048q]   8 accumulated d-chunk matmuls/block
  kT = wk.T @ xT          [128h, 4096k]
  vT = wv.T @ xT          [128h, 4096k] -> v natural [k,h] via PE transposes
  per q-half (1024 q), per key block kb (128 keys):
    scT = kT[:,kb].T @ qT [128k, 1024q]   (PSUM)
    ex  = exp(scT)        (ACT, PSUM->SBUF, float32r)
    quad-reduce ex blocks on DVE; den += ones.T @ quad  (denominator
      replicated across partitions by the all-ones stationary matrix)
    oT  += v[kb].T' @ ex  [128h, 1024q]   (PSUM accumulate over kb)
  outT = oT * reciprocal_approx(den)  -> DMA out

float32r = fp32 bytes with reduced-precision PE multiply at full PE rate
(4x faster than fp32 matmul); measured output error vs float64 reference
~2.3e-4 relative to absmax.
"""

import sys

sys.path.insert(0, "/opt/trn_rl_repo")

import numpy as np

import concourse.mybir as mybir
from concourse import bacc
from concourse.bass_utils import run_bass_kernel_spmd
from concourse.masks import make_identity
from concourse.tile import TileContext

F32 = mybir.dt.float32
MD = mybir.dt.float32r

B = 4  # batch
D = 1024  # d_model
H = 128  # head size
S = 4096  # full sequence (keys)
SQ = 2048  # queries per core
DC = D // 128  # 8 d-chunks
NB = S // 512  # 8 column blocks for projections
KB = S // 128  # 32 key blocks
TREE = 4  # densum quad-reduction on DVE
SC_BUFS = 4
EXP_BUFS = 8
XT_BUFS = 24


def _build_attention():
    nc = bacc.Bacc("TRN2", target_bir_lowering=False, debug=False)

    xT = nc.dram_tensor("xT", (D, S), MD, kind="ExternalInput")
    wq = nc.dram_tensor("wq", (D, H), MD, kind="ExternalInput")
    wk = nc.dram_tensor("wk", (D, H), MD, kind="ExternalInput")
    wv = nc.dram_tensor("wv", (D, H), MD, kind="ExternalInput")
    outT = nc.dram_tensor("outT", (H, SQ), F32, kind="ExternalOutput")

    with TileContext(nc) as tc:
        with (
            tc.tile_pool(name="consts", bufs=1) as cpool,
            tc.tile_pool(name="big", bufs=1) as big,
            tc.tile_pool(name="xtp", bufs=XT_BUFS) as xtp,
            tc.tile_pool(name="expp", bufs=EXP_BUFS) as expp,
            tc.tile_pool(name="work", bufs=2) as work,
            tc.tile_pool(name="ps", bufs=1, space="PSUM") as ps,
        ):
            # ---- constants ----
            wq_sb = cpool.tile([128, DC, H], MD)
            nc.sync.dma_start(out=wq_sb, in_=wq.rearrange("(c p) h -> p c h", p=128))
            wk_sb = cpool.tile([128, DC, H], MD)
            nc.sync.dma_start(out=wk_sb, in_=wk.rearrange("(c p) h -> p c h", p=128))
            wv_sb = cpool.tile([128, DC, H], MD)
            nc.sync.dma_start(out=wv_sb, in_=wv.rearrange("(c p) h -> p c h", p=128))
            ones_f32 = cpool.tile([128, 128], F32)
            nc.vector.memset(ones_f32, 1.0)
            ones_sb = cpool.tile([128, 128], MD)
            nc.vector.tensor_copy(out=ones_sb, in_=ones_f32)
            ident = cpool.tile([128, 128], F32)
            make_identity(nc, ident)

            # ---- persistent activations ----
            qT_sb = big.tile([128, SQ], MD)
            kT_sb = big.tile([128, S], MD)
            v_sb = big.tile([128, S], MD)  # col block kb = v[128 keys, 128 h]
            vT_sb = big.tile([128, S], F32)

            # ---- phase 1: projections (PSUM->SBUF copies on idle ACT) ----
            for nb in range(NB):
                xts = []
                for dc in range(DC):
                    xt_t = xtp.tile([128, 512], MD, tag="xt", name=f"xt_{nb}_{dc}")
                    nc.sync.dma_start(
                        out=xt_t,
                        in_=xT[dc * 128 : (dc + 1) * 128, nb * 512 : (nb + 1) * 512],
                    )
                    xts.append(xt_t)
                cols = slice(nb * 512, (nb + 1) * 512)

                kps = ps.tile([128, 512], F32, tag="sc", bufs=SC_BUFS, name=f"kps{nb}")
                for dc in range(DC):
                    nc.tensor.matmul(
                        kps, wk_sb[:, dc], xts[dc], start=dc == 0, stop=dc == DC - 1
                    )
                nc.scalar.copy(out=kT_sb[:, cols], in_=kps)

                vps = ps.tile([128, 512], F32, tag="sc", bufs=SC_BUFS, name=f"vps{nb}")
                for dc in range(DC):
                    nc.tensor.matmul(
                        vps, wv_sb[:, dc], xts[dc], start=dc == 0, stop=dc == DC - 1
                    )
                nc.scalar.copy(out=vT_sb[:, cols], in_=vps)

                if nb < SQ // 512:
                    qps = ps.tile(
                        [128, 512], F32, tag="sc", bufs=SC_BUFS, name=f"qps{nb}"
                    )
                    for dc in range(DC):
                        nc.tensor.matmul(
                            qps,
                            wq_sb[:, dc],
                            xts[dc],
                            start=dc == 0,
                            stop=dc == DC - 1,
                        )
                    nc.scalar.copy(out=qT_sb[:, cols], in_=qps)

                # v natural: transpose the four 128x128 blocks of vT[:, nb]
                for t in range(4):
                    blk = slice(nb * 512 + t * 128, nb * 512 + (t + 1) * 128)
                    vtp = ps.tile(
                        [128, 512], F32, tag="sc", bufs=SC_BUFS, name=f"vtp{nb}_{t}"
                    )
                    nc.tensor.transpose(vtp[:, 0:128], vT_sb[:, blk], ident)
                    nc.scalar.copy(out=v_sb[:, blk], in_=vtp[:, 0:128])

            # ---- phase 2: attention ----
            for sqh in range(SQ // 1024):
                qc = sqh * 1024
                oT_ps = ps.tile([128, 1024], F32, tag="oT", bufs=1, name=f"oT{sqh}")
                den_ps = ps.tile([128, 1024], F32, tag="den", bufs=1, name=f"den{sqh}")
                pend = []
                n_groups = KB // TREE

                def emit_scores_exp(kb, sqh=sqh, qc=qc):
                    kcol = slice(kb * 128, (kb + 1) * 128)
                    scs = []
                    for c in range(2):
                        sc_t = ps.tile(
                            [128, 512],
                            F32,
                            tag="sc",
                            bufs=SC_BUFS,
                            name=f"sc{sqh}_{kb}_{c}",
                        )
                        nc.tensor.matmul(
                            sc_t,
                            kT_sb[:, kcol],
                            qT_sb[:, qc + c * 512 : qc + (c + 1) * 512],
                            start=True,
                            stop=True,
                        )
                        scs.append(sc_t)
                    ex = expp.tile([128, 1024], MD, tag="ex", name=f"ex{sqh}_{kb}")
                    for c in range(2):
                        nc.scalar.activation(
                            ex[:, c * 512 : (c + 1) * 512],
                            scs[c],
                            mybir.ActivationFunctionType.Exp,
                        )
                    return ex

                def emit_consumers(kb, ex, sqh=sqh, oT_ps=oT_ps, den_ps=den_ps):
                    # denominator: quad-reduce on DVE, then all-ones matmul
                    pend.append(ex)
                    if len(pend) == TREE:
                        gi = kb // TREE
                        t1 = expp.tile(
                            [128, 1024], MD, tag="ex", name=f"r1_{sqh}_{gi}"
                        )
                        nc.vector.tensor_add(t1, pend[0], pend[1])
                        t2 = expp.tile(
                            [128, 1024], MD, tag="ex", name=f"r2_{sqh}_{gi}"
                        )
                        nc.vector.tensor_add(t2, pend[2], pend[3])
                        red = expp.tile(
                            [128, 1024], MD, tag="ex", name=f"red{sqh}_{gi}"
                        )
                        nc.vector.tensor_add(red, t1, t2)
                        for c in range(2):
                            cc = slice(c * 512, (c + 1) * 512)
                            nc.tensor.matmul(
                                den_ps[:, cc],
                                ones_sb,
                                red[:, cc],
                                start=gi == 0,
                                stop=gi == n_groups - 1,
                            )
                        pend.clear()
                    # output accumulation
                    kcol = slice(kb * 128, (kb + 1) * 128)
                    for c in range(2):
                        cc = slice(c * 512, (c + 1) * 512)
                        nc.tensor.matmul(
                            oT_ps[:, cc],
                            v_sb[:, kcol],
                            ex[:, cc],
                            start=kb == 0,
                            stop=kb == KB - 1,
                        )

                # software pipeline: consumers of ex(kb) emitted after
                # scores(kb+2) so PE never waits on ACT
                exs = {}
                for kb in range(KB + 2):
                    if kb < KB:
                        exs[kb] = emit_scores_exp(kb)
                    if kb >= 2:
                        emit_consumers(kb - 2, exs.pop(kb - 2))

                recip = work.tile([128, 1024], F32, tag="recip", name=f"recip{sqh}")
                nc.vector.reciprocal_approx_fast(out=recip, in_=den_ps)
                onrm = work.tile([128, 1024], F32, tag="onrm", name=f"onrm{sqh}")
                nc.vector.tensor_mul(onrm, oT_ps, recip)
                nc.sync.dma_start(out=outT[:, qc : qc + 1024], in_=onrm)

    nc.compile()
    return nc


_NC_CACHE = None


def _get_nc():
    global _NC_CACHE
    if _NC_CACHE is None:
        _NC_CACHE = _build_attention()
    return _NC_CACHE


def kernel(x, Wq, Wk, Wv):
    x = np.asarray(x, dtype=np.float32)
    Wq = np.asarray(Wq, dtype=np.float32)
    Wk = np.asarray(Wk, dtype=np.float32)
    Wv = np.asarray(Wv, dtype=np.float32)
    assert x.shape == (B, S, D), x.shape

    wq = np.ascontiguousarray(Wq / np.sqrt(np.float32(H)))
    wk = np.ascontiguousarray(Wk)
    wv = np.ascontiguousarray(Wv)
    in_maps = []
    for c in range(8):
        bi, hq = divmod(c, 2)
        xt = x[bi].T  # [d, s]
        if hq == 1:
            xt = np.concatenate([xt[:, SQ:], xt[:, :SQ]], axis=1)
        in_maps.append(
            {"xT": np.ascontiguousarray(xt), "wq": wq, "wk": wk, "wv": wv}
        )

    nc = _get_nc()
    res = run_bass_kernel_spmd(nc, in_maps, core_ids=list(range(8)))

    out = np.empty((B, S, H), dtype=np.float32)
    for c in range(8):
        bi, hq = divmod(c, 2)
        out[bi, hq * SQ : (hq + 1) * SQ] = res.results[c]["outT"].T
    return out


if __name__ == "__main__":
    # quick self-test with random inputs
    rng = np.random.default_rng(0)
    x = rng.standard_normal((B, S, D), dtype=np.float32)
    s = 1.0 / np.sqrt(D)
    Wq = rng.standard_normal((D, H), dtype=np.float32) * s
    Wk = rng.standard_normal((D, H), dtype=np.float32) * s
    Wv = rng.standard_normal((D, H), dtype=np.float32) * s
    out = kernel(x=x, Wq=Wq, Wk=Wk, Wv=Wv)
    print("out", out.shape, out.dtype, float(np.abs(out).max()))

